# revision 3
# baseline (speedup 1.0000x reference)
"""Multi-head causal self-attention (B=2, T=4096, D=768, H=12) on 8 trn2 cores.

Sharding: core c -> batch b = c//4, heads 3*(c%4) .. 3*(c%4)+2.
qkv_proj column-parallel (each core computes Q/K/V only for its heads),
out_proj row-parallel (each core emits a partial y^T; host sums the 4
partials per batch).

Device dataflow (all fp32):
  x^T tiles via PE transposes -> Q^T/K^T via transposed projection
  (W^T stationary, x^T streaming) -> S^T = K Q^T in [k,q] layout ->
  exp on ScalarE (no max subtraction; scores ~ N(0,1)) -> causal mask on
  the diagonal band -> out^T = V^T_hat @ P^T with a ones row appended to V
  so the softmax denominators fall out of the same matmul -> scale by
  reciprocal -> y^T = Wo^T.T @ out^T accumulated over local heads.
"""

import sys

sys.path.insert(0, "/opt/trn_rl_repo")

import numpy as np
from contextlib import ExitStack

import concourse.bass as bass
import concourse.bacc as bacc
import concourse.tile as tile
import concourse.mybir as mybir
from concourse.masks import make_identity
from concourse.bass_utils import run_bass_kernel_spmd

F32 = mybir.dt.float32
AF = mybir.ActivationFunctionType

B = 2
T = 4096
D = 768
H = 12
DK = 64
NCORES = 8
HL = 3  # heads per core
ND = D // 128  # 6 d-tiles
NKT = T // 128  # 32 k-tiles
NQB = T // 512  # 8 q-blocks
NTSB = T // 512  # 8 t-superblocks (4 t-tiles each)

_CACHE = {}


def _emit(tc):
    nc = tc.nc
    x_d = nc.dram_tensor("x", [T, D], F32, kind="ExternalInput").ap()
    wqk_d = nc.dram_tensor("wqkT", [D, 384], F32, kind="ExternalInput").ap()
    wv_d = nc.dram_tensor("wvT", [D, HL * DK], F32, kind="ExternalInput").ap()
    wo_d = nc.dram_tensor("woT", [HL, DK, D], F32, kind="ExternalInput").ap()
    y_d = nc.dram_tensor("yT", [D, T], F32, kind="ExternalOutput").ap()

    ctx = ExitStack()
    const = ctx.enter_context(tc.tile_pool(name="const", bufs=1))
    persist = ctx.enter_context(tc.tile_pool(name="persist", bufs=1))
    xpool = ctx.enter_context(tc.tile_pool(name="xp", bufs=2))
    xtpool = ctx.enter_context(tc.tile_pool(name="xt", bufs=1))
    ptpool = ctx.enter_context(tc.tile_pool(name="pt", bufs=6))
    spool = ctx.enter_context(tc.tile_pool(name="sp", bufs=2))
    otpool = ctx.enter_context(tc.tile_pool(name="ot", bufs=1))
    ypool = ctx.enter_context(tc.tile_pool(name="yp", bufs=2))
    psS = ctx.enter_context(tc.tile_pool(name="psS", bufs=2, space="PSUM"))
    psO = ctx.enter_context(tc.tile_pool(name="psO", bufs=3, space="PSUM"))
    psM = ctx.enter_context(tc.tile_pool(name="psM", bufs=1, space="PSUM"))

    # ---- constants ----
    ident = const.tile([128, 128], F32)
    make_identity(nc, ident)
    # upper-triangular (incl diag) 0/1 mask: keep where q >= k
    tri = const.tile([128, 128], F32)
    nc.gpsimd.memset(tri, 1.0)
    nc.gpsimd.affine_select(
        out=tri, in_=tri, compare_op=mybir.AluOpType.is_ge, fill=0.0,
        base=0, pattern=[[1, 128]], channel_multiplier=-1,
    )
    ones64 = const.tile([1, 64], F32)
    nc.vector.memset(ones64, 1.0)

    wqk_sb = const.tile([128, ND, 384], F32)
    nc.sync.dma_start(out=wqk_sb, in_=wqk_d.rearrange("(j p) e -> p j e", p=128))
    wv_sb = const.tile([128, ND, HL * DK], F32)
    nc.sync.dma_start(out=wv_sb, in_=wv_d.rearrange("(j p) e -> p j e", p=128))
    wo_sb = const.tile([DK, HL, D], F32)
    nc.sync.dma_start(out=wo_sb, in_=wo_d.rearrange("h p d -> p h d"))

    # ---- persistent activations ----
    # KA: [K^T_h0 ; K^T_h1], QB: [Q^T_h0 ; Q^T_h1] on partition halves
    KA = persist.tile([128, T], F32, name="KA")
    QB = persist.tile([128, T], F32, name="QB")
    C2 = persist.tile([128, T], F32, name="C2")  # [K^T_h2 ; Q^T_h2]
    D2 = persist.tile([128, T], F32, name="D2")  # [Q^T_h2 ; K^T_h2] (swapped copy)
    Vh = []
    for h in range(HL):
        vt = persist.tile([128, NKT, DK + 1], F32, name=f"V{h}")
        nc.gpsimd.memset(vt[:, :, DK : DK + 1], 1.0)  # ones column -> softmax sums
        Vh.append(vt)

    qk_dest = [KA, QB, C2]

    # ================= phase A: projections =================
    for tsb in range(NTSB):
        xt_sb = xtpool.tile([128, ND, 512], F32, name="xt_sb")
        for tt in range(4):
            t0 = (tsb * 4 + tt) * 128
            x_sb = xpool.tile([128, D], F32, name="x_sb")
            nc.sync.dma_start(out=x_sb, in_=x_d[t0 : t0 + 128, :])
            ps_t = psS.tile([128, ND * 128], F32, name="ps_t", tag="s")
            for dj in range(ND):
                nc.tensor.transpose(
                    ps_t[:, dj * 128 : (dj + 1) * 128],
                    x_sb[:, dj * 128 : (dj + 1) * 128],
                    ident,
                )
            nc.vector.tensor_copy(
                xt_sb[:, :, tt * 128 : (tt + 1) * 128],
                ps_t.rearrange("p (j t) -> p j t", j=ND),
            )
        # Q^T / K^T projection: out[e, t] block per e-tile
        for et in range(3):
            ps_q = psS.tile([128, 512], F32, name="ps_q", tag="s")
            for dj in range(ND):
                nc.tensor.matmul(
                    ps_q,
                    lhsT=wqk_sb[:, dj, et * 128 : (et + 1) * 128],
                    rhs=xt_sb[:, dj, :],
                    start=(dj == 0),
                    stop=(dj == ND - 1),
                )
            nc.vector.tensor_copy(qk_dest[et][:, tsb * 512 : (tsb + 1) * 512], ps_q)
        # D2 = partition-swapped copy of C2 (for self-paired row-tiling of h2)
        blk = slice(tsb * 512, (tsb + 1) * 512)
        nc.sync.dma_start(out=D2[0:64, blk], in_=C2[64:128, blk])
        nc.sync.dma_start(out=D2[64:128, blk], in_=C2[0:64, blk])
        # V natural: stationary x^T tiles, streaming Wv^T
        for tt in range(4):
            ps_v = psM.tile([128, HL * DK], F32, name="ps_v", tag="m")
            for dj in range(ND):
                nc.tensor.matmul(
                    ps_v,
                    lhsT=xt_sb[:, dj, tt * 128 : (tt + 1) * 128],
                    rhs=wv_sb[:, dj, :],
                    start=(dj == 0),
                    stop=(dj == ND - 1),
                )
            kt = tsb * 4 + tt
            for h in range(HL):
                nc.vector.tensor_copy(
                    Vh[h][:, kt, 0:DK], ps_v[:, h * DK : (h + 1) * DK]
                )

    # ================= phase B: attention =================
    for qb in range(NQB):
        nk = 4 * (qb + 1)
        qblk = slice(qb * 512, (qb + 1) * 512)
        ot_tiles = [None] * HL
        # pass 0: heads (0, 1) row-paired; pass 1: head 2 self-paired
        for hpass, heads in enumerate([(0, 1), (2,)]):
            pso = {}
            for h in heads:
                pso[h] = psO.tile([DK + 1, 512], F32, name=f"pso{h}", tag="o")
            for kp in range(nk // 2):
                kt0, kt1 = 2 * kp, 2 * kp + 1
                ssb = {}
                for h in heads:
                    ssb[h] = psS.tile([128, 1024], F32, name=f"ssb{h}", tag="s")
                for i, kt in enumerate((kt0, kt1)):
                    kblk = slice(kt * 128, (kt + 1) * 128)
                    off = i * 512
                    if hpass == 0:
                        nc.tensor.matmul(
                            ssb[0][:, off : off + 512],
                            lhsT=KA[0:64, kblk], rhs=QB[0:64, qblk],
                            start=True, stop=True,
                        )
                        nc.tensor.matmul(
                            ssb[1][:, off : off + 512],
                            lhsT=KA[64:128, kblk], rhs=QB[64:128, qblk],
                            start=True, stop=True,
                        )
                    else:
                        if i == 0:
                            nc.tensor.matmul(
                                ssb[2][:, 0:512],
                                lhsT=C2[0:64, kblk], rhs=D2[0:64, qblk],
                                start=True, stop=True,
                            )
                        else:
                            nc.tensor.matmul(
                                ssb[2][:, 512:1024],
                                lhsT=D2[64:128, kblk], rhs=C2[64:128, qblk],
                                start=True, stop=True,
                            )
                for h in heads:
                    pt = ptpool.tile([128, 1024], F32, name="pt")
                    nc.scalar.activation(pt, ssb[h], AF.Exp, scale=0.125)
                    for i, kt in enumerate((kt0, kt1)):
                        off = i * 512
                        if kt >= 4 * qb:  # diagonal band tile
                            bp = kt - 4 * qb
                            if bp > 0:
                                nc.gpsimd.memset(pt[:, off : off + bp * 128], 0.0)
                            dg = slice(off + bp * 128, off + (bp + 1) * 128)
                            nc.vector.tensor_mul(pt[:, dg], pt[:, dg], tri)
                        nc.tensor.matmul(
                            pso[h],
                            lhsT=Vh[h][:, kt, :],
                            rhs=pt[:, off : off + 512],
                            start=(kt == 0),
                            stop=(kt == nk - 1),
                        )
            # normalize: out^T[0:64] * (1 / sums) ; sums are row 64
            for h in heads:
                sums_sb = spool.tile([1, 512], F32, name="sums_sb")
                nc.vector.tensor_copy(sums_sb, pso[h][DK : DK + 1, :])
                chop = spool.tile([128, 4], F32, name="chop")
                nc.sync.dma_start(out=chop, in_=sums_sb)
                recipC = spool.tile([128, 4], F32, name="recipC")
                nc.vector.reciprocal(recipC, chop)
                recipR = spool.tile([1, 512], F32, name="recipR")
                nc.sync.dma_start(out=recipR, in_=recipC)
                psb = psM.tile([64, 512], F32, name="psb", tag="m")
                nc.tensor.matmul(psb, lhsT=ones64, rhs=recipR, start=True, stop=True)
                recipb = spool.tile([64, 512], F32, name="recipb")
                nc.vector.tensor_copy(recipb, psb)
                ot = otpool.tile([64, 512], F32, name=f"ot{h}", tag=f"ot{h}")
                nc.vector.tensor_mul(ot, pso[h][0:DK, :], recipb)
                ot_tiles[h] = ot
        # out-proj: y^T[d, q] accumulated over the 3 local heads
        for dj in range(ND):
            ps_y = psM.tile([128, 512], F32, name="ps_y", tag="m")
            for h in range(HL):
                nc.tensor.matmul(
                    ps_y,
                    lhsT=wo_sb[:, h, dj * 128 : (dj + 1) * 128],
                    rhs=ot_tiles[h],
                    start=(h == 0),
                    stop=(h == HL - 1),
                )
            y_sb = ypool.tile([128, 512], F32, name="y_sb")
            nc.vector.tensor_copy(y_sb, ps_y)
            nc.sync.dma_start(out=y_d[dj * 128 : (dj + 1) * 128, qblk], in_=y_sb)
    ctx.close()


def build():
    if "nc" in _CACHE:
        return _CACHE["nc"]
    nc = bacc.Bacc(
        "TRN2", target_bir_lowering=False, debug=False, num_devices=NCORES
    )
    with tile.TileContext(nc) as tc:
        _emit(tc)
    nc.compile()
    _CACHE["nc"] = nc
    return nc


def make_in_maps(x, w_qkv, w_out):
    x = np.asarray(x, dtype=np.float32)
    w_qkv = np.asarray(w_qkv, dtype=np.float32)
    w_out = np.asarray(w_out, dtype=np.float32)
    wq = w_qkv[0:D]        # [768, 768], rows = q features
    wk = w_qkv[D : 2 * D]
    wv = w_qkv[2 * D :]
    in_maps = []
    for c in range(NCORES):
        b, g = divmod(c, 4)
        hs = [3 * g + j for j in range(HL)]  # global head ids
        h0, h1, h2 = hs
        cols = []
        for pair in ((wk, h0), (wk, h1), (wq, h0), (wq, h1), (wk, h2), (wq, h2)):
            w, h = pair
            cols.append(w[h * DK : (h + 1) * DK].T)  # [768, 64]
        wqkT = np.ascontiguousarray(np.concatenate(cols, axis=1))  # [768, 384]
        wvT = np.ascontiguousarray(
            np.concatenate([wv[h * DK : (h + 1) * DK].T for h in hs], axis=1)
        )  # [768, 192]
        woT = np.ascontiguousarray(
            np.stack([w_out[:, h * DK : (h + 1) * DK].T for h in hs])
        )  # [3, 64, 768]
        in_maps.append(
            {
                "x": np.ascontiguousarray(x[b]),
                "wqkT": wqkT,
                "wvT": wvT,
                "woT": woT,
            }
        )
    return in_maps


def run(inputs, trace=False):
    """Run on hardware; returns (y [B,T,D] fp32, BassKernelResults)."""
    nc = build()
    in_maps = make_in_maps(inputs["x"], inputs["w_qkv"], inputs["w_out"])
    br = run_bass_kernel_spmd(nc, in_maps, list(range(NCORES)), trace=trace)
    y = np.zeros((B, T, D), dtype=np.float32)
    for c in range(NCORES):
        b = c // 4
        y[b] += np.asarray(br.results[c]["yT"]).T
    return y, br


def kernel(x, w_qkv, w_out):
    y, _ = run({"x": x, "w_qkv": w_qkv, "w_out": w_out})
    return y
